# revision 1
# baseline (speedup 1.0000x reference)
"""CountNCELoss Trainium2 kernel.

Computes: scores s[p,b,n,l] = c[b,l,:] @ W[p] @ z[b,(l+shift)%L,:]
          loss = sum_{p,b,l<len[b]} (logsumexp_n s - s_pos) / sum(len)

Strategy: 128-row l-blocks, only blocks with l < length[b] are computed,
round-robin balanced over 8 cores. Per block: PE computes Wc = c_blk @ W[p]
(12 matmuls, PSUM), ACT copies to SBUF bf16, DVE tensor_tensor_reduce does
the 108 shifted row-dot products, logsumexp on ACT/DVE, masked accumulate
via ACT scale+accum. Host pre-transposes c and pre-gathers shifted z rows.
"""

import json
import math
import os

import numpy as np
import ml_dtypes

import concourse.bass as bass
import concourse.bass_utils as bass_utils
import concourse.bass2jax as bass2jax
import concourse.mybir as mybir
import concourse.tile as tile
from concourse.bass_utils import run_bass_kernel_spmd
from concourse.vector_clock import ScopedClock, VectorClock

# ---------------------------------------------------------------------------
# Workarounds for the walrus build in this container, which only accepts ONE
# sem-wait per instruction. Tile attaches several (multi-producer deps and the
# kernel-tail drain). Fix A: split multi-wait instructions in the BIR JSON
# into single-wait EventSemaphore carriers. Fix B: emit the kernel-tail drain
# as one single-wait drain per logical proc.
# ---------------------------------------------------------------------------

_orig_compile_bir_kernel = bass_utils.compile_bir_kernel


def _split_multiwait(bir_json: bytes) -> bytes:
    bir = json.loads(bir_json)
    ctr = 0
    changed = False
    for fn in bir.get("functions", []):
        for blk in fn.get("blocks", []):
            out = []
            for inst in blk.get("instructions", []):
                si = inst.get("sync_info") or {}
                waits = si.get("on_wait") or []
                if len(waits) > 1:
                    changed = True
                    for w in waits[:-1]:
                        ctr += 1
                        out.append(
                            {
                                "debug": inst.get("debug", 0),
                                "engine": inst["engine"],
                                "ins": [],
                                "outs": [],
                                "name": f"{inst['name']}_xw{ctr}",
                                "opcode": "EventSemaphore",
                                "sync_info": {"on_update": [], "on_wait": [w]},
                            }
                        )
                    si["on_wait"] = [waits[-1]]
                out.append(inst)
            blk["instructions"] = out
    if not changed:
        return bir_json
    return json.dumps(bir).encode()


def _patched_compile_bir_kernel(bir_json, tmpdir, neff_name="file.neff"):
    if isinstance(bir_json, str):
        bir_json = bir_json.encode()
    return _orig_compile_bir_kernel(_split_multiwait(bir_json), tmpdir, neff_name)


if bass_utils.compile_bir_kernel is not _patched_compile_bir_kernel:
    bass_utils.compile_bir_kernel = _patched_compile_bir_kernel
    bass2jax.compile_bir_kernel = _patched_compile_bir_kernel


def _drain_and_barrier_single_wait(self, tick_clock, wait_clock):
    gc = tick_clock.global_clock
    n = len(gc)
    for proc in range(n):
        t = gc[proc]
        if t == 0:
            continue
        vc = VectorClock([t if i == proc else 0 for i in range(n)])
        d = self.nc.sync.drain()
        wait_clock.add_sem_waits(d.ins, ScopedClock({None: vc}))
    self.nc.all_engine_barrier()
    popped = self.nc._tile_sem_poison_stack.pop()
    assert popped is self._sem_poison
    self.nc.clear_and_free_semaphores(list(self.sems.allocated().values()))
    self.nc.all_engine_barrier()


tile.TileContext._drain_and_barrier = _drain_and_barrier_single_wait

BF16 = ml_dtypes.bfloat16

B, L, CD, ZD = 32, 1024, 256, 256
P, NEG = 12, 8
NCORES = 8
BLK = 128
NSL = P + NEG  # 20 z-slices per block: 12 pos shifts (p+1), 8 neg shifts
NSHIFT = 9  # scores per p: 1 pos + 8 neg

_prog_cache = {}

# dot-stage engine assignment (per p index, same every block):
# muls on GPSIMD for these p's (rest on DVE); reduces on ACT for these p's
# (rest: DVE halving cascade).
GPS_MUL_PS = frozenset(int(x) for x in os.environ.get("K_GPS_MUL", "0,2,5,8,10").split(",") if x != "")
ACT_RED_PS = frozenset(int(x) for x in os.environ.get("K_ACT_RED", "1,4,7").split(",") if x != "")
CFG = {
    "psum_split": True,   # two [128,6,256] PSUM tiles with bufs=2 (vs one 12-tile bufs=1)
    "mask_on_dve": True,  # mask-mul + p-sum on DVE instead of ACT copy+accum
    "z_bufs": 3,
    "prod_bufs": 3,
    "wc_bufs": 2,
}

# set by kernel() after a traced run
last_exec_ns = None
last_profile = None


def _build_program(nblk, reps=1):
    """One SPMD program; all cores run it with their own data.

    reps>1 replays the block loop (same data) for differential timing."""
    nc = bass.Bass()
    f32 = mybir.dt.float32
    bf16 = mybir.dt.bfloat16

    cT_d = nc.declare_dram_parameter("cT", [nblk, 128, 2, 128], bf16, isOutput=False)
    z9_d = nc.declare_dram_parameter("z9", [nblk, 128, NSL, ZD], bf16, isOutput=False)
    W_d = nc.declare_dram_parameter("Wt", [128, 2, P, ZD], bf16, isOutput=False)
    mask_d = nc.declare_dram_parameter("maskc", [128, nblk], f32, isOutput=False)
    out_d = nc.declare_dram_parameter("part", [128, 1], f32, isOutput=True)

    psum_split = CFG["psum_split"]
    with tile.TileContext(nc) as tc:
        with (
            tc.tile_pool(name="const", bufs=1) as const_pool,
            tc.tile_pool(name="cblk", bufs=3) as cpool,
            tc.tile_pool(name="zblk", bufs=CFG["z_bufs"]) as zpool,
            tc.tile_pool(name="wcsb", bufs=CFG["wc_bufs"]) as wcpool,
            tc.tile_pool(
                name="wcps", bufs=(2 if psum_split else 1), space="PSUM"
            ) as pspool,
            tc.tile_pool(name="prod", bufs=CFG["prod_bufs"]) as prodpool,
            tc.tile_pool(name="lse", bufs=4) as lpool,
            tc.tile_pool(name="fin", bufs=1) as fpool,
        ):
            w_t = const_pool.tile([128, 2, P, ZD], bf16)
            nc.sync.dma_start(out=w_t, in_=W_d[:])
            mask_t = const_pool.tile([128, nblk], f32)
            nc.sync.dma_start(out=mask_t, in_=mask_d[:])
            blkacc = const_pool.tile([128, nblk], f32)
            nc.vector.memset(blkacc, 0.0)

            for _rep in range(reps):
              for i in range(nblk):
                ct = cpool.tile([128, 2, 128], bf16)
                nc.sync.dma_start(out=ct, in_=cT_d[i])
                z9t = zpool.tile([128, NSL, ZD], bf16)
                nc.sync.dma_start(out=z9t, in_=z9_d[i])

                # Wc[p] = c_blk @ W[p] -> PSUM, then PSUM -> SBUF bf16 (ACT).
                # NOTE: accumulation group (start..stop) must be contiguous
                # per PSUM region — interleaved groups compute wrong results.
                # rhs pairs two p's -> [128, 512] per matmul (1 PSUM bank)
                wc_sb = wcpool.tile([128, P, ZD], bf16)
                if psum_split:
                    PG = P // 2
                    for g in range(2):
                        wc_ps = pspool.tile([128, PG, ZD], f32)
                        for pp in range(0, PG, 2):
                            for h in range(2):
                                nc.tensor.matmul(
                                    wc_ps[:, pp : pp + 2, :],
                                    lhsT=ct[:, h, :],
                                    rhs=w_t[:, h, g * PG + pp : g * PG + pp + 2, :],
                                    start=(h == 0),
                                    stop=(h == 1),
                                )
                        nc.scalar.copy(
                            wc_sb[:, g * PG : (g + 1) * PG, :], wc_ps
                        )
                else:
                    wc_ps = pspool.tile([128, P, ZD], f32)
                    for p in range(0, P, 2):
                        for h in range(2):
                            nc.tensor.matmul(
                                wc_ps[:, p : p + 2, :],
                                lhsT=ct[:, h, :],
                                rhs=w_t[:, h, p : p + 2, :],
                                start=(h == 0),
                                stop=(h == 1),
                            )
                    for h in range(2):
                        nc.scalar.copy(
                            wc_sb[:, h * (P // 2) : (h + 1) * (P // 2), :],
                            wc_ps[:, h * (P // 2) : (h + 1) * (P // 2), :],
                        )

                # 108 shifted dot products -> scores [128, P, 9]
                # per p: products for [pos, 8 negs] then reduce over d.
                # muls split DVE/GPSIMD, reduces split DVE-cascade/ACT-accum.
                scores = lpool.tile([128, P, NSHIFT], f32, tag="scores")
                for p in range(P):
                    mul_eng = nc.gpsimd if p in GPS_MUL_PS else nc.vector
                    tag = "prodg" if p in GPS_MUL_PS else f"prodv{p % 2}"
                    prod = prodpool.tile([128, NSHIFT, ZD], bf16, tag=tag)
                    wc_p = wc_sb[:, p, :]
                    # pos shift product -> slot 0
                    mul_eng.tensor_tensor(
                        out=prod[:, 0, :],
                        in0=wc_p,
                        in1=z9t[:, p, :],
                        op=mybir.AluOpType.mult,
                    )
                    # neg shifts products -> slots 1..8 (broadcast wc over 8)
                    wc_b = bass.AP(
                        tensor=wc_p.tensor,
                        offset=wc_p.offset,
                        ap=[wc_p.ap[0], [0, NEG], wc_p.ap[1]],
                    )
                    mul_eng.tensor_tensor(
                        out=prod[:, 1:, :],
                        in0=wc_b,
                        in1=z9t[:, P:, :],
                        op=mybir.AluOpType.mult,
                    )
                    if p in ACT_RED_PS:
                        # ACT: 9 fused copy+accum ops
                        for j in range(NSHIFT):
                            ascr = prodpool.tile([128, ZD], bf16, tag="ascr")
                            nc.scalar.activation(
                                out=ascr,
                                in_=prod[:, j, :],
                                func=mybir.ActivationFunctionType.Copy,
                                bias=0.0,
                                scale=1.0,
                                accum_out=scores[:, p, j : j + 1],
                            )
                    else:
                        # DVE: halving cascade in bf16, then 1x reduce
                        c1 = prodpool.tile([128, NSHIFT, ZD // 2], bf16, tag="c1")
                        nc.vector.tensor_tensor(
                            out=c1,
                            in0=prod[:, :, : ZD // 2],
                            in1=prod[:, :, ZD // 2 :],
                            op=mybir.AluOpType.add,
                        )
                        c2 = prodpool.tile([128, NSHIFT, ZD // 4], bf16, tag="c2")
                        nc.vector.tensor_tensor(
                            out=c2,
                            in0=c1[:, :, : ZD // 4],
                            in1=c1[:, :, ZD // 4 :],
                            op=mybir.AluOpType.add,
                        )
                        c3 = prodpool.tile([128, NSHIFT, ZD // 8], bf16, tag="c3")
                        nc.vector.tensor_tensor(
                            out=c3,
                            in0=c2[:, :, : ZD // 8],
                            in1=c2[:, :, ZD // 8 :],
                            op=mybir.AluOpType.add,
                        )
                        nc.vector.tensor_reduce(
                            out=scores[:, p, :],
                            in_=c3,
                            axis=mybir.AxisListType.X,
                            op=mybir.AluOpType.add,
                        )

                # logsumexp over the 9 shifts, per p
                nmax = lpool.tile([128, P], f32, tag="nmax")
                nc.vector.tensor_reduce(
                    out=nmax,
                    in_=scores,
                    axis=mybir.AxisListType.X,
                    op=mybir.AluOpType.max,
                    negate=True,
                )
                sumexp = lpool.tile([128, P], f32, tag="sumexp")
                for p in range(P):
                    escr = prodpool.tile([128, NSHIFT], f32, tag="escr")
                    nc.scalar.activation(
                        out=escr,
                        in_=scores[:, p, :],
                        func=mybir.ActivationFunctionType.Exp,
                        bias=nmax[:, p : p + 1],
                        scale=1.0,
                        accum_out=sumexp[:, p : p + 1],
                    )
                lse0 = lpool.tile([128, P], f32, tag="lse0")
                nc.scalar.activation(
                    out=lse0,
                    in_=sumexp,
                    func=mybir.ActivationFunctionType.Ln,
                )
                # loss[l,p] = lse - s_pos = (lse0 - nmax) - s_pos
                d1 = lpool.tile([128, P], f32, tag="d1")
                nc.vector.tensor_tensor(
                    out=d1, in0=lse0, in1=nmax, op=mybir.AluOpType.subtract
                )
                d2 = lpool.tile([128, P], f32, tag="d2")
                nc.vector.tensor_tensor(
                    out=d2, in0=d1, in1=scores[:, :, 0], op=mybir.AluOpType.subtract
                )
                # blkacc[:, i] = sum_p mask[l] * d2[l, p]
                if CFG["mask_on_dve"]:
                    d3 = lpool.tile([128, P], f32, tag="d3")
                    nc.vector.tensor_scalar_mul(d3, d2, mask_t[:, i : i + 1])
                    nc.vector.tensor_reduce(
                        out=blkacc[:, i : i + 1],
                        in_=d3,
                        axis=mybir.AxisListType.X,
                        op=mybir.AluOpType.add,
                    )
                else:
                    mscr = lpool.tile([128, P], f32, tag="mscr")
                    nc.scalar.activation(
                        out=mscr,
                        in_=d2,
                        func=mybir.ActivationFunctionType.Copy,
                        bias=0.0,
                        scale=mask_t[:, i : i + 1],
                        accum_out=blkacc[:, i : i + 1],
                    )

            accf = fpool.tile([128, 1], f32)
            nc.vector.tensor_reduce(
                out=accf,
                in_=blkacc,
                axis=mybir.AxisListType.X,
                op=mybir.AluOpType.add,
            )
            nc.sync.dma_start(out=out_d[:], in_=accf)

    return nc


def prepare(c, z, W, neg_shift, length):
    """Host-side sharding/layout. Returns (in_maps, nblk, msum)."""
    c = np.asarray(c, dtype=np.float32)
    z = np.asarray(z, dtype=np.float32)
    W = np.asarray(W, dtype=np.float32)
    neg_shift = np.asarray(neg_shift).astype(np.int64)
    length = np.asarray(length).astype(np.int64)

    shifts = np.concatenate([np.arange(1, P + 1), neg_shift]).astype(np.int64)  # [20]

    # Every (b, l<length[b]) row is independent: pack all valid rows densely
    # into 128-row blocks (across b boundaries), balanced over cores.
    bs = np.concatenate([np.full(int(length[b]), b, np.int64) for b in range(B)])
    ls = np.concatenate([np.arange(int(length[b]), dtype=np.int64) for b in range(B)])
    total = bs.shape[0]
    total_blocks = (total + BLK - 1) // BLK
    nblk = (total_blocks + NCORES - 1) // NCORES
    padded = nblk * NCORES * BLK
    valid = np.zeros(padded, dtype=bool)
    valid[:total] = True
    bs = np.concatenate([bs, np.zeros(padded - total, np.int64)])
    ls = np.concatenate([ls, np.zeros(padded - total, np.int64)])
    # row r of block k of core g <- global packed row ((k*NCORES)+g)*128 + r
    order = (
        (np.arange(nblk)[:, None] * NCORES + np.arange(NCORES)[None, :]) * BLK
    )  # [nblk, NCORES] block starts

    # Wt[cc, h, p, :] = W[p, h*128+cc, :]
    Wt = np.ascontiguousarray(
        W.reshape(P, 2, 128, ZD).transpose(2, 1, 0, 3)
    ).astype(BF16)

    lidx = np.arange(BLK)
    in_maps = []
    for core in range(NCORES):
        idx = order[:, core][:, None] + lidx[None, :]  # [nblk, 128] global rows
        bv = bs[idx]  # [nblk, 128]
        lv = ls[idx]
        mk = valid[idx]
        # cT[i, cc, h, l] = c[bv, lv, h*128+cc]
        cfull = c[bv, lv]  # [nblk, 128, 256]
        cT_arr = np.ascontiguousarray(
            cfull.reshape(nblk, BLK, 2, 128).transpose(0, 3, 2, 1)
        ).astype(BF16)
        rows = (lv[:, :, None] + shifts[None, None, :]) % L  # [nblk, 128, NSL]
        z9_arr = z[bv[:, :, None], rows].astype(BF16)  # [nblk, 128, NSL, ZD]
        mask_arr = np.ascontiguousarray(mk.T).astype(np.float32)  # [128, nblk]
        in_maps.append(
            {"cT": cT_arr, "z9": z9_arr, "Wt": Wt, "maskc": mask_arr}
        )
    return in_maps, nblk, float(length.sum())


def kernel(c, z, W, neg_shift, length):
    global last_exec_ns, last_profile
    in_maps, nblk, msum = prepare(c, z, W, neg_shift, length)

    if nblk not in _prog_cache:
        _prog_cache[nblk] = _build_program(nblk)
    nc = _prog_cache[nblk]

    # the device occasionally throws a transient NRT_EXEC_UNIT_UNRECOVERABLE
    # on a fresh NEFF load; retry a couple of times
    res = None
    for attempt in range(3):
        try:
            res = run_bass_kernel_spmd(nc, in_maps, core_ids=list(range(NCORES)))
            break
        except Exception:
            if attempt == 2:
                raise
    last_exec_ns = res.exec_time_ns
    last_profile = res.profile_json

    total = 0.0
    for r in res.results:
        total += r["part"].astype(np.float64).sum()
    return np.array(total / msum, dtype=np.float32)

